# revision 1
# baseline (speedup 1.0000x reference)
"""EntitySelector sparse-attention kernel for 8 Trainium2 NeuronCores.

Sharding: data-parallel over batch (16 batches -> 2 per core). Each core:
  - gathers its entities from the (replicated, HBM-resident) ent_emb table
    via indirect DMA (only 256 rows/batch move, never the full 80MB table),
  - runs Q/K projections, masked softmax attention, WO projection and
    LayerNorm entirely on-core (no collectives),
  - all matmuls in fp32r (full-rate PE, ~10-bit-mantissa rounding) except
    the value path (probs @ K and @ WO) which runs in bf16.

Layout strategy: activations are kept feature-major ([D, L] / [D, NB]) so
every matmul contracts over the partition dim; softmax runs on [L, NB]
tiles (reductions along the free dim); PE-transposes bridge the two.
"""

import sys

sys.path.insert(0, "/opt/trn_rl_repo")

import numpy as np
import ml_dtypes

import concourse.bass as bass
import concourse.mybir as mybir
import concourse.tile as tile
from concourse.tile_rust import add_dep_helper
from concourse import bacc
from concourse.bass_utils import run_bass_kernel_spmd
from concourse.masks import make_identity

P = 128
D = 1024
DT = D // P            # 8 feature tiles
B = 16
BL = 2                 # batches per core
L = 1024
LC = 512               # l-chunk (psum free dim)
NLC = L // LC          # 2 chunks
LT = LC // P           # 4 l-tiles per chunk
NB = 256
NT = NB // P           # 2 entity tiles
NE = 20000
NCORES = 8

F32 = mybir.dt.float32
F32R = mybir.dt.float32r
BF16 = mybir.dt.bfloat16
I32 = mybir.dt.int32

AF = mybir.ActivationFunctionType
OP = mybir.AluOpType
AX = mybir.AxisListType

_CACHE = {}


class _Ctx:
    pass


def _emit_gather(nc, g, b):
    """Indirect-gather this batch's entities (issue early: SWDGE ring FIFO)."""
    idx_col = g.bpool.tile([P, NT], I32, tag="idxc")
    nc.gpsimd.dma_start(idx_col, g.idx[b].rearrange("(t p) -> p t", p=P))
    msk_col = g.bpool.tile([P, NT], F32, tag="mskc")
    nc.gpsimd.dma_start(msk_col, g.msk[b].rearrange("(t p) -> p t", p=P))
    msk_bc = g.bpool.tile([P, NB], BF16, tag="mskb")
    mrow = g.msk[b]
    nc.gpsimd.dma_start(
        msk_bc,
        bass.AP(tensor=mrow.tensor, offset=mrow.offset,
                ap=[[0, P]] + list(mrow.ap)))

    ent_sb = []
    g_insts = []
    for nt in range(NT):
        e = g.entp.tile([P, D], F32, tag="ent")
        gi = nc.gpsimd.indirect_dma_start(
            out=e[:], out_offset=None, in_=g.emb[:, :],
            in_offset=bass.IndirectOffsetOnAxis(ap=idx_col[:, nt:nt + 1], axis=0))
        g_insts.append(gi)
        nc.gpsimd.tensor_scalar_mul(e[:], e[:], msk_col[:, nt:nt + 1])
        ent_sb.append(e)
    return ent_sb, msk_bc, g_insts


def _emit_batch(nc, g, b, ent_sb, msk_bc):
    """Emit one batch's compute pipeline."""
    # ---- entT (feature-major masked entities, fp32r) ----
    entT = g.enttp.tile([P, DT, NB], F32R, tag="entT")
    for dt in range(DT):
        pt = g.ps_tr.tile([P, NB], F32, tag="ptr")
        for nt in range(NT):
            nc.tensor.transpose(pt[:, nt * P:(nt + 1) * P],
                                ent_sb[nt][:, dt * P:(dt + 1) * P], g.ident)
        nc.vector.tensor_copy(entT[:, dt, :], pt)

    # ---- KT = (ent @ WK^T + bk), feature-major [dout, n] ----
    kt_sb = g.ktp.tile([P, DT, NB], F32R, tag="kt")
    for do in range(DT):
        pk = g.ps_sc.tile([P, NB], F32, tag="psc")
        for kt in range(DT):
            nc.tensor.matmul(pk, g.wk_sb[:, kt, do * P:(do + 1) * P],
                             entT[:, kt, :],
                             start=(kt == 0), stop=(kt == DT - 1))
        nc.vector.tensor_scalar_add(kt_sb[:, do, :], pk, g.bk_col[:, do:do + 1])

    # ---- K entity-major (bf16 value path) ----
    k_sb = g.kp.tile([P, NT, D], BF16, tag="k")
    for nt in range(NT):
        for dt in range(0, DT, 2):
            pt = g.ps_tr.tile([P, NB], F32R, tag="ptr")
            for j in range(2):
                nc.tensor.transpose(pt[:, j * P:(j + 1) * P],
                                    kt_sb[:, dt + j, nt * P:(nt + 1) * P],
                                    g.ident_r)
            nc.vector.tensor_copy(k_sb[:, nt, dt * P:(dt + 2) * P], pt)

    qTb = g.qT[b].rearrange("(kt p) l -> p kt l", p=P)
    for lc in range(NLC):
        _emit_chunk(nc, g, b, lc, qTb, kt_sb, k_sb, msk_bc)


def _emit_chunk(nc, g, b, lc, qTb, kt_sb, k_sb, msk_bc):
    # ---- load transposed query chunk ----
    H = DT // 2
    qin = g.qinp.tile([P, DT, LC], F32R, tag="qin")
    qin_i = nc.sync.dma_start(qin, qTb[:, :, lc * LC:(lc + 1) * LC])
    if b == 0 and getattr(g, "g0_insts", None):
        for gi in g.g0_insts:
            add_dep_helper(qin_i.ins, gi.ins,
                           reason="query stream after entity gather")
    if b == 0 and lc == 1 and getattr(g, "wk_insts", None):
        for wi in g.wk_insts:
            add_dep_helper(qin_i.ins, wi.ins,
                           reason="2nd query chunk after K weights")

    # ---- Q projection (feature-major; two half-tiles loosen deps) ----
    qt_sb = [g.qtp.tile([P, H, LC], F32R, tag=f"qt{i}", name=f"qt{i}") for i in range(2)]
    for do in range(DT):
        pq = g.ps_big.tile([P, LC], F32, tag="pbig")
        for kt in range(DT):
            nc.tensor.matmul(pq, g.wq_sb[:, kt, do * P:(do + 1) * P],
                             qin[:, kt, :],
                             start=(kt == 0), stop=(kt == DT - 1))
        nc.vector.tensor_scalar_add(qt_sb[do // H][:, do % H, :], pq,
                                    g.bq_col[:, do:do + 1])

    # ---- scores + masked softmax + transpose ----
    probsT = g.ptp.tile([P, NT, LC], BF16, tag="pT")
    for t in range(LT):
        _emit_softmax_tile(nc, g, t, qt_sb, kt_sb, msk_bc, probsT)

    # ---- PV (bf16): pvT[dout, l]; two half-tiles loosen deps ----
    pvT = [g.pvp.tile([P, H, LC], BF16, tag=f"pv{i}", name=f"pv{i}") for i in range(2)]
    for do in range(DT):
        pp = g.ps_big.tile([P, LC], F32, tag="pbig")
        for nt in range(NT):
            nc.tensor.matmul(pp, k_sb[:, nt, do * P:(do + 1) * P],
                             probsT[:, nt, :],
                             start=(nt == 0), stop=(nt == NT - 1))
        nc.vector.tensor_copy(pvT[do // H][:, do % H, :], pp)

    # ---- WO + bias + LayerNorm, per l-tile ----
    for t in range(LT):
        _emit_out_tile(nc, g, b, lc * LT + t, t, pvT)


def _emit_softmax_tile(nc, g, t, qt_sb, kt_sb, msk_bc, probsT):
    psc = g.ps_sc.tile([P, NB], F32, tag="psc")
    H = DT // 2
    for dt in range(DT):
        nc.tensor.matmul(psc, qt_sb[dt // H][:, dt % H, t * P:(t + 1) * P],
                         kt_sb[:, dt, :],
                         start=(dt == 0), stop=(dt == DT - 1))
    negmax = g.lnp.tile([P, 1], F32, tag="nm")
    nc.vector.reduce_max(negmax, psc, axis=AX.X, negate=True)
    probs = g.probsp.tile([P, NB], BF16, tag="probs")
    nc.scalar.activation(out=probs, in_=psc, func=AF.Exp, bias=negmax, scale=1.0)
    nc.gpsimd.tensor_mul(probs, probs, msk_bc)
    rsum = g.lnp.tile([P, 1], F32, tag="rs")
    nc.vector.reduce_sum(rsum, probs, axis=AX.X)
    rinv = g.lnp.tile([P, 1], F32, tag="ri")
    nc.vector.reciprocal(rinv, rsum)
    nc.scalar.mul(rinv, rinv, float(D) ** -0.5)
    nc.gpsimd.tensor_scalar_mul(probs, probs, rinv)
    ptb = g.ps_tr.tile([P, NB], BF16, tag="ptr")
    for nt in range(NT):
        nc.tensor.transpose(ptb[:, nt * P:(nt + 1) * P],
                            probs[:, nt * P:(nt + 1) * P], g.ident_b)
    nc.vector.tensor_copy(probsT[:, :, t * P:(t + 1) * P],
                          ptb.rearrange("p (a b) -> p a b", a=NT))


def _emit_out_tile(nc, g, b, lt, t, pvT):
    H = DT // 2
    o_sb = [g.opool.tile([P, LC], F32, tag=f"o{i}", name=f"o{i}")
            for i in range(2)]
    stats = g.lnp.tile([P, 2, 6], F32, tag="stats")
    for half in range(2):
        po = g.ps_big.tile([P, LC], F32, tag="pbig")
        for dt in range(DT):
            nc.tensor.matmul(po, pvT[dt // H][:, dt % H, t * P:(t + 1) * P],
                             g.wo_sb[:, dt, half * LC:(half + 1) * LC],
                             start=(dt == 0), stop=(dt == DT - 1))
        nc.vector.tensor_add(o_sb[half], po, g.bo_bc[:, half * LC:(half + 1) * LC])
        nc.vector.bn_stats(out=stats[:, half, :], in_=o_sb[half])

    mv = g.lnp.tile([P, 2], F32, tag="mv")
    nc.vector.bn_aggr(out=mv, in_=stats)
    rstd = g.lnp.tile([P, 1], F32, tag="rstd")
    nc.scalar.activation(out=rstd, in_=mv[:, 1:2], func=AF.Sqrt,
                         bias=g.eps_t, scale=1.0)
    nc.vector.reciprocal(rstd, rstd)
    for half in range(2):
        nc.gpsimd.tensor_scalar(out=o_sb[half], in0=o_sb[half],
                                scalar1=mv[:, 0:1], scalar2=rstd,
                                op0=OP.subtract, op1=OP.mult)
        if g.apply_affine:
            nc.vector.tensor_mul(o_sb[half], o_sb[half],
                                 g.lng_bc[:, half * LC:(half + 1) * LC])
            nc.vector.tensor_add(o_sb[half], o_sb[half],
                                 g.lnb_bc[:, half * LC:(half + 1) * LC])
        nc.scalar.dma_start(
            g.out[b, lt * P:(lt + 1) * P, half * LC:(half + 1) * LC],
            o_sb[half])


def build_nc(apply_affine):
    nc = bacc.Bacc("TRN2", target_bir_lowering=False, debug=False,
                   num_devices=NCORES)
    g = _Ctx()
    g.apply_affine = apply_affine

    g.qT = nc.dram_tensor("qT", [BL, D, L], F32R, kind="ExternalInput")
    g.emb = nc.dram_tensor("emb", [NE, D], F32, kind="ExternalInput")
    g.idx = nc.dram_tensor("idx", [BL, NB], I32, kind="ExternalInput")
    g.msk = nc.dram_tensor("msk", [BL, NB], F32, kind="ExternalInput")
    wq = nc.dram_tensor("wq", [D, D], F32R, kind="ExternalInput")
    wk = nc.dram_tensor("wk", [D, D], F32R, kind="ExternalInput")
    wo = nc.dram_tensor("wo", [D, D], BF16, kind="ExternalInput")
    bq = nc.dram_tensor("bq", [D], F32, kind="ExternalInput")
    bk = nc.dram_tensor("bk", [D], F32, kind="ExternalInput")
    bo = nc.dram_tensor("bo", [D], F32, kind="ExternalInput")
    if apply_affine:
        lng = nc.dram_tensor("lng", [D], F32, kind="ExternalInput")
        lnb = nc.dram_tensor("lnb", [D], F32, kind="ExternalInput")
    g.out = nc.dram_tensor("out", [BL, L, D], F32, kind="ExternalOutput")

    def bcast_row(dram_1d):
        ap = dram_1d[:]
        return bass.AP(tensor=ap.tensor, offset=ap.offset,
                       ap=[[0, P]] + list(ap.ap))

    with tile.TileContext(nc) as tc:
        with (
            tc.tile_pool(name="wpool", bufs=1) as wpool,
            tc.tile_pool(name="bpool", bufs=2) as bpool,
            tc.tile_pool(name="entp", bufs=2) as entp,
            tc.tile_pool(name="entt", bufs=1) as enttp,
            tc.tile_pool(name="ktp", bufs=1) as ktp,
            tc.tile_pool(name="kp", bufs=1) as kp,
            tc.tile_pool(name="qinp", bufs=2) as qinp,
            tc.tile_pool(name="qtp", bufs=1) as qtp,
            tc.tile_pool(name="probsp", bufs=4) as probsp,
            tc.tile_pool(name="ptp", bufs=2) as ptp,
            tc.tile_pool(name="pvp", bufs=2) as pvp,
            tc.tile_pool(name="op", bufs=3) as opool,
            tc.tile_pool(name="lnp", bufs=4) as lnp,
            tc.tile_pool(name="ps_big", bufs=4, space="PSUM") as ps_big,
            tc.tile_pool(name="ps_sc", bufs=3, space="PSUM") as ps_sc,
            tc.tile_pool(name="ps_tr", bufs=1, space="PSUM") as ps_tr,
        ):
            g.bpool, g.entp, g.enttp, g.ktp, g.kp = bpool, entp, enttp, ktp, kp
            g.qinp, g.qtp, g.probsp, g.ptp, g.pvp = qinp, qtp, probsp, ptp, pvp
            g.opool, g.lnp = opool, lnp
            g.ps_big, g.ps_sc, g.ps_tr = ps_big, ps_sc, ps_tr

            g.ident = wpool.tile([P, P], F32)
            make_identity(nc, g.ident)
            g.ident_r = wpool.tile([P, P], F32R)
            nc.vector.tensor_copy(g.ident_r, g.ident)
            g.ident_b = wpool.tile([P, P], BF16)
            nc.vector.tensor_copy(g.ident_b, g.ident)
            g.eps_t = wpool.tile([P, 1], F32)
            nc.vector.memset(g.eps_t, 1e-5)

            # batch-0 gathers go first on the SWDGE ring
            ent0, mskbc0, g.g0_insts = _emit_gather(nc, g, 0)

            g.wq_sb = wpool.tile([P, DT, D], F32R)
            wq_r = wq[:, :].rearrange("(kt p) m -> p kt m", p=P)
            g.wk_sb = wpool.tile([P, DT, D], F32R)
            wk_r = wk[:, :].rearrange("(kt p) m -> p kt m", p=P)
            h = DT // 2
            bulk = []
            bulk.append(nc.scalar.dma_start(g.wk_sb[:, :h, :], wk_r[:, :h, :]))
            bulk.append(nc.scalar.dma_start(g.wk_sb[:, h:, :], wk_r[:, h:, :]))
            g.wk_insts = list(bulk)
            bulk.append(nc.scalar.dma_start(g.wq_sb[:, :h, :], wq_r[:, :h, :]))
            bulk.append(nc.scalar.dma_start(g.wq_sb[:, h:, :], wq_r[:, h:, :]))
            g.wo_sb = wpool.tile([P, DT, D], BF16)
            wo_i = nc.gpsimd.dma_start(g.wo_sb,
                                wo[:, :].rearrange("(kt p) m -> p kt m", p=P))
            for bi in bulk:
                for gi in g.g0_insts:
                    add_dep_helper(bi.ins, gi.ins,
                                   reason="bulk weight load after entity gather")
            add_dep_helper(wo_i.ins, bulk[-1].ins,
                           reason="wo load after wq (needed late)")

            g.bq_col = wpool.tile([P, DT], F32)
            nc.gpsimd.dma_start(g.bq_col, bq[:].rearrange("(t p) -> p t", p=P))
            g.bk_col = wpool.tile([P, DT], F32)
            nc.gpsimd.dma_start(g.bk_col, bk[:].rearrange("(t p) -> p t", p=P))
            g.bo_bc = wpool.tile([P, D], BF16)
            nc.gpsimd.dma_start(g.bo_bc, bcast_row(bo))

            if apply_affine:
                g.lng_bc = wpool.tile([P, D], F32)
                nc.gpsimd.dma_start(g.lng_bc, bcast_row(lng))
                g.lnb_bc = wpool.tile([P, D], F32)
                nc.gpsimd.dma_start(g.lnb_bc, bcast_row(lnb))

            ent1, mskbc1, _ = _emit_gather(nc, g, 1)
            gathers = [(ent0, mskbc0), (ent1, mskbc1)]
            for b in range(BL):
                ent_sb, msk_bc = gathers[b]
                _emit_batch(nc, g, b, ent_sb, msk_bc)

    nc.compile()
    return nc


def _get_nc(apply_affine):
    key = bool(apply_affine)
    if key not in _CACHE:
        _CACHE[key] = build_nc(key)
    return _CACHE[key]


def kernel(query, ent_emb, ent_idx_in_batch, max_entity_number,
           WQ_w, WQ_b, WK_w, WK_b, WO_w, WO_b, ln_g, ln_b):
    query = np.asarray(query, np.float32)
    ent_emb = np.ascontiguousarray(np.asarray(ent_emb, np.float32))
    idx = np.asarray(ent_idx_in_batch)
    mask = (idx != -1).astype(np.float32)
    idx32 = np.where(idx < 0, 0, idx).astype(np.int32)
    wq = np.ascontiguousarray(np.asarray(WQ_w, np.float32).T)
    wk = np.ascontiguousarray(np.asarray(WK_w, np.float32).T)
    wo = np.ascontiguousarray(np.asarray(WO_w, np.float32).T).astype(
        ml_dtypes.bfloat16)
    bq = np.ascontiguousarray(np.asarray(WQ_b, np.float32))
    bk = np.ascontiguousarray(np.asarray(WK_b, np.float32))
    bo = np.ascontiguousarray(np.asarray(WO_b, np.float32))
    lng = np.asarray(ln_g, np.float32)
    lnb = np.asarray(ln_b, np.float32)
    apply_affine = not (np.all(lng == 1.0) and np.all(lnb == 0.0))

    qT = np.ascontiguousarray(query.transpose(0, 2, 1))  # (B, D, L)

    nc = _get_nc(apply_affine)
    in_maps = []
    for c in range(NCORES):
        s = slice(c * BL, (c + 1) * BL)
        m = dict(
            qT=np.ascontiguousarray(qT[s]),
            emb=ent_emb,
            idx=np.ascontiguousarray(idx32[s]),
            msk=np.ascontiguousarray(mask[s]),
            wq=wq, wk=wk, wo=wo, bq=bq, bk=bk, bo=bo,
        )
        if apply_affine:
            m["lng"] = lng
            m["lnb"] = lnb
        in_maps.append(m)

    res = run_bass_kernel_spmd(nc, in_maps, core_ids=list(range(NCORES)))
    return np.concatenate([r["out"] for r in res.results], axis=0)

